# revision 1
# baseline (speedup 1.0000x reference)
"""Embedding lookup kernel for Trainium2 (8 NeuronCores).

Problem: x [1, 8192] int token ids, weights [49408, 768] f32 table
         -> out [8192, 768] f32  (out[s] = weights[x[0, s]])

Strategy: data-parallel over the sequence. Each of the 8 cores gets the
full table (resident in its HBM) plus a 1024-token slice of ids, and
gathers its 1024 rows via SWDGE indirect DMA (one 3KB descriptor per
row), bouncing through SBUF, then writes its [1024, 768] output slice.
Host concatenates the 8 slices.
"""

import numpy as np

import concourse.bacc as bacc
import concourse.bass as bass
import concourse.mybir as mybir
import concourse.tile as tile
from concourse.bass_utils import run_bass_kernel_spmd

SEQ = 8192
VOCAB = 49408
DIM = 768
NCORES = 8
P = 128

TOK = SEQ // NCORES  # tokens per core
TPP = 1  # tokens per partition per gather tile
TILES = TOK // (P * TPP)

_cache = {}


def _build(tok=TOK, tpp=TPP):
    """Build the per-core Bass program: gather `tok` rows of the table."""
    tiles = tok // (P * tpp)
    nc = bacc.Bacc(None, dynamic_dma_scratch_size=max(16384, P * tpp * 64 * 2))

    ids = nc.dram_tensor("ids", [tiles, P, tpp], mybir.dt.int32, kind="ExternalInput")
    weights = nc.dram_tensor(
        "weights", [VOCAB, DIM], mybir.dt.float32, kind="ExternalInput"
    )
    out = nc.dram_tensor("out", [tok, DIM], mybir.dt.float32, kind="ExternalOutput")
    # out viewed as [tiles, P, tpp*DIM]: token t*P*tpp + p*tpp + j <-> (t, p, j)
    out_t = out.rearrange("(t p k) d -> t p (k d)", p=P, k=tpp)

    with tile.TileContext(nc) as tc:
        with tc.tile_pool(name="sbuf", bufs=4) as sbuf:
            for t in range(tiles):
                idx_tile = sbuf.tile([P, tpp], mybir.dt.int32, tag="idx")
                g_tile = sbuf.tile([P, tpp * DIM], mybir.dt.float32, tag="gather")
                nc.sync.dma_start(out=idx_tile[:], in_=ids[t])
                nc.gpsimd.indirect_dma_start(
                    out=g_tile[:],
                    out_offset=None,
                    in_=weights[:],
                    in_offset=bass.IndirectOffsetOnAxis(ap=idx_tile[:, :tpp], axis=0),
                )
                nc.sync.dma_start(out=out_t[t], in_=g_tile[:])

    nc.compile()
    return nc


def _get_nc():
    key = (TOK, TPP)
    if key not in _cache:
        _cache[key] = _build()
    return _cache[key]


def _run(x, weights, trace=False):
    ids = np.ascontiguousarray(np.asarray(x).reshape(-1).astype(np.int32))
    w = np.ascontiguousarray(np.asarray(weights, dtype=np.float32))
    assert ids.shape == (SEQ,) and w.shape == (VOCAB, DIM)

    nc = _get_nc()
    in_maps = [
        {
            "ids": ids[c * TOK : (c + 1) * TOK].reshape(TILES, P, TPP),
            "weights": w,
        }
        for c in range(NCORES)
    ]
    br = run_bass_kernel_spmd(nc, in_maps, list(range(NCORES)), trace=trace)
    out = np.concatenate([br.results[c]["out"] for c in range(NCORES)], axis=0)
    return out, br


def kernel(x, weights):
    out, _ = _run(x, weights)
    return out


# revision 5
# speedup vs baseline: 1.1333x; 1.1333x over previous
"""Embedding lookup kernel for Trainium2 (8 NeuronCores).

Problem: x [1, 8192] int token ids, weights [49408, 768] f32 table
         -> out [8192, 768] f32  (out[s] = weights[x[0, s]])

Strategy: data-parallel over the sequence. Each of the 8 cores gets the
full table (resident in its HBM) plus a 1024-token slice of ids, and
gathers its 1024 rows via SWDGE indirect DMA (one 3KB descriptor per
row), bouncing through SBUF, then writes its [1024, 768] output slice.
Host concatenates the 8 slices.

Raw Bass (no TileContext): the Tile framework's preamble + EVSEM drain
tail cost ~16us on a ~20us kernel. Manual semaphores: one 4KB id load,
then TILES gathers on gpsimd each followed by an output store on sync,
every buffer resident in SBUF so nothing ever stalls on reuse.
"""

import numpy as np

import concourse.bass as bass
import concourse.mybir as mybir
from concourse.bass_utils import run_bass_kernel_spmd

SEQ = 8192
VOCAB = 49408
DIM = 768
NCORES = 8
P = 128

TOK = SEQ // NCORES  # tokens per core
TILES = TOK // P  # gather tiles of P tokens

_cache = {}


def _build():
    """Per-core program: out[t*P+p, :] = weights[ids[p, t], :].

    ids arrives host-transposed as [P, TILES] (column t = tile t's P
    tokens) so a single contiguous 4KB DMA loads every index.
    """
    nc = bass.Bass()

    ids = nc.dram_tensor("ids", [P, TILES], mybir.dt.int32, kind="ExternalInput")
    weights = nc.dram_tensor(
        "weights", [VOCAB, DIM], mybir.dt.float32, kind="ExternalInput"
    )
    out = nc.dram_tensor("out", [TOK, DIM], mybir.dt.float32, kind="ExternalOutput")
    out_t = out.rearrange("(t p) d -> t p d", p=P)  # [TILES, P, DIM]

    import contextlib

    with contextlib.ExitStack() as ctx:
        idx_sb = ctx.enter_context(nc.sbuf_tensor([P, TILES], mybir.dt.int32))
        g_sb = ctx.enter_context(nc.sbuf_tensor([P, TILES * DIM], mybir.dt.float32))
        idx_sem = ctx.enter_context(nc.semaphore("idx_sem"))
        store_sem = ctx.enter_context(nc.semaphore("store_sem"))
        # one sem per gather: DMA sem updates may not cross other DMAs'
        # waited values (race detector), so don't share a counter
        gather_sems = [
            ctx.enter_context(nc.semaphore(f"gather_sem{t}")) for t in range(TILES)
        ]
        block = ctx.enter_context(nc.Block())

        @block.sync
        def _(sync: bass.BassEngine):
            sync.dma_start(out=idx_sb[:], in_=ids[:]).then_inc(idx_sem, 16)
            for t in range(TILES):
                sync.wait_ge(gather_sems[t], 16)
                sync.dma_start(
                    out=out_t[t], in_=g_sb[:, t * DIM : (t + 1) * DIM]
                ).then_inc(store_sem, 16)
            sync.wait_ge(store_sem, 16 * TILES)

        @block.gpsimd
        def _(gpsimd: bass.BassEngine):
            gpsimd.wait_ge(idx_sem, 16)
            for t in range(TILES):
                gpsimd.indirect_dma_start(
                    out=g_sb[:, t * DIM : (t + 1) * DIM],
                    out_offset=None,
                    in_=weights[:],
                    in_offset=bass.IndirectOffsetOnAxis(
                        ap=idx_sb[:, t : t + 1], axis=0
                    ),
                ).then_inc(gather_sems[t], 16)

    return nc


def _get_nc():
    if "nc" not in _cache:
        _cache["nc"] = _build()
    return _cache["nc"]


def _run(x, weights, trace=False):
    ids = np.ascontiguousarray(np.asarray(x).reshape(-1).astype(np.int32))
    w = np.ascontiguousarray(np.asarray(weights, dtype=np.float32))
    assert ids.shape == (SEQ,) and w.shape == (VOCAB, DIM)

    nc = _get_nc()
    in_maps = [
        {
            # token t*P+p of this core's slice lands at idx_sb[p, t]
            "ids": np.ascontiguousarray(
                ids[c * TOK : (c + 1) * TOK].reshape(TILES, P).T
            ),
            "weights": w,
        }
        for c in range(NCORES)
    ]
    br = run_bass_kernel_spmd(nc, in_maps, list(range(NCORES)), trace=trace)
    out = np.concatenate([br.results[c]["out"] for c in range(NCORES)], axis=0)
    return out, br


def kernel(x, weights):
    out, _ = _run(x, weights)
    return out
